# revision 1
# baseline (speedup 1.0000x reference)
"""Linear dynamical system (Kalman-style forward simulation) on 8 TRN2 cores.

Sharding: data-parallel over batch (64 -> 8 per core). Per core, the T=2048
sequential scan is restructured as NCH=32 independent chunks of C=64 steps:
  phase A: batched local scan over all chunks at once (zero init), fused with
           the input-drive GEMM u = X @ M in the same PSUM accumulation group.
  phase B: sequential carry propagation across chunk boundaries via A^C.
  phase C: batched correction scan (R_i = R_{i-1} A) added into local states,
           plus the observation GEMM obs.T = H @ states.T + sqrt_S_V @ Vn.T.
All device tensors live feature-major (transposed); the host pre/post-permutes.
"""
import numpy as np
import concourse.bass as bass
import concourse.mybir as mybir
import concourse.tile as tile
from concourse import bacc
from concourse.bass_utils import run_bass_kernel_spmd

B, T, S = 64, 2048, 256
NCORE, B_loc, NCH, C = 8, 8, 32, 64
NCOLS = NCH * C * B_loc  # 16384
FP = mybir.dt.float32

_nc_cache = []
TRACE = False
last_result = []


def _build():
    nc = bacc.Bacc("TRN2", target_bir_lowering=False, debug=False)
    xt_d = nc.dram_tensor("xt", [512, NCOLS], FP, kind="ExternalInput").ap()
    vnt_d = nc.dram_tensor("vnt", [S, NCOLS], FP, kind="ExternalInput").ap()
    s0t_d = nc.dram_tensor("s0t", [S, B_loc], FP, kind="ExternalInput").ap()
    m_d = nc.dram_tensor("mmat", [512, S], FP, kind="ExternalInput").ap()
    a_d = nc.dram_tensor("amat", [S, S], FP, kind="ExternalInput").ap()
    ac_d = nc.dram_tensor("acmat", [S, S], FP, kind="ExternalInput").ap()
    ht_d = nc.dram_tensor("htmat", [S, S], FP, kind="ExternalInput").ap()
    svt_d = nc.dram_tensor("svtmat", [S, S], FP, kind="ExternalInput").ap()
    st_d = nc.dram_tensor("statesT", [S, NCOLS], FP, kind="ExternalOutput").ap()
    ob_d = nc.dram_tensor("obsT", [S, NCOLS], FP, kind="ExternalOutput").ap()

    with tile.TileContext(nc) as tc:
        with (
            tc.tile_pool(name="const", bufs=1) as cpool,
            tc.tile_pool(name="Lp", bufs=1) as lpool,
            tc.tile_pool(name="xt", bufs=8) as xpool,
            tc.tile_pool(name="rp", bufs=3) as rpool,
            tc.tile_pool(name="vp", bufs=2) as vpool,
            tc.tile_pool(name="op", bufs=2) as opool,
            tc.tile_pool(name="ps", bufs=2, space="PSUM") as pspool,
        ):
            def load_blocks(src, kparts, name):
                out = []
                for k in range(kparts):
                    row = []
                    for h in range(2):
                        t_ = cpool.tile([128, 128], FP, tag=f"{name}{k}{h}", name=f"{name}{k}{h}")
                        nc.sync.dma_start(
                            t_[:], src[k * 128:(k + 1) * 128, h * 128:(h + 1) * 128]
                        )
                        row.append(t_)
                    out.append(row)
                return out

            Mt = load_blocks(m_d, 4, "M")
            At = load_blocks(a_d, 2, "A")
            ACt = load_blocks(ac_d, 2, "AC")
            Ht = load_blocks(ht_d, 2, "H")
            SVt = load_blocks(svt_d, 2, "SV")

            Lt = [lpool.tile([128, NCOLS], FP, tag=f"L{h}", name=f"L{h}") for h in range(2)]
            et = [lpool.tile([128, NCH * B_loc], FP, tag=f"ET{h}", name=f"ET{h}") for h in range(2)]

            # ---- phase A: u GEMM + batched local scan fused in PSUM ----
            for i in range(C):
                cols = slice(i * 256, (i + 1) * 256)
                pcols = slice((i - 1) * 256, i * 256)
                xts = []
                for k in range(4):
                    xt = xpool.tile([128, 256], FP, tag="xt")
                    nc.sync.dma_start(xt[:], xt_d[k * 128:(k + 1) * 128, cols])
                    xts.append(xt)
                for h in range(2):
                    pu = pspool.tile([128, 256], FP, tag=f"pu{h}")
                    for k in range(4):
                        nc.tensor.matmul(
                            pu[:], Mt[k][h][:], xts[k][:],
                            start=(k == 0), stop=(i == 0 and k == 3),
                            skip_group_check=True,
                        )
                    if i > 0:
                        for k in range(2):
                            nc.tensor.matmul(
                                pu[:], At[k][h][:], Lt[k][:, pcols],
                                start=False, stop=(k == 1),
                                skip_group_check=True,
                            )
                    if h == 0:
                        nc.scalar.copy(Lt[h][:, cols], pu[:])
                    else:
                        nc.vector.tensor_copy(out=Lt[h][:, cols], in_=pu[:])

            # ---- phase B: sequential carries across chunks ----
            for h in range(2):
                nc.sync.dma_start(et[h][:, 0:B_loc], s0t_d[h * 128:(h + 1) * 128, :])
            for c in range(1, NCH):
                o = (C - 1) * 256 + (c - 1) * B_loc
                for h in range(2):
                    pe = pspool.tile([128, B_loc], FP, tag=f"pu{h}")
                    for k in range(2):
                        nc.tensor.matmul(
                            pe[:], ACt[k][h][:], et[k][:, (c - 1) * B_loc:c * B_loc],
                            start=(k == 0), stop=(k == 1), skip_group_check=True,
                        )
                    nc.vector.tensor_add(
                        out=et[h][:, c * B_loc:(c + 1) * B_loc],
                        in0=pe[:], in1=Lt[h][:, o:o + B_loc],
                    )

            # ---- phase C: correction scan + states out + obs GEMM ----
            Rcur = et
            for i in range(C):
                cols = slice(i * 256, (i + 1) * 256)
                Rnew = []
                for h in range(2):
                    pr = pspool.tile([128, 256], FP, tag=f"pu{h}")
                    for k in range(2):
                        nc.tensor.matmul(
                            pr[:], At[k][h][:], Rcur[k][:],
                            start=(k == 0), stop=(k == 1), skip_group_check=True,
                        )
                    rn = rpool.tile([128, 256], FP, tag=f"R{h}")
                    nc.scalar.copy(rn[:], pr[:])
                    nc.vector.tensor_add(
                        out=Lt[h][:, cols], in0=Lt[h][:, cols], in1=pr[:]
                    )
                    nc.sync.dma_start(st_d[h * 128:(h + 1) * 128, cols], Lt[h][:, cols])
                    Rnew.append(rn)
                Rcur = Rnew
                if i % 2 == 1:
                    bcols = slice((i - 1) * 256, (i + 1) * 256)
                    vns = []
                    for k in range(2):
                        vn = vpool.tile([128, 512], FP, tag=f"vn{k}")
                        nc.sync.dma_start(vn[:], vnt_d[k * 128:(k + 1) * 128, bcols])
                        vns.append(vn)
                    for h in range(2):
                        po = pspool.tile([128, 512], FP, tag=f"po{h}")
                        for k in range(2):
                            nc.tensor.matmul(
                                po[:], SVt[k][h][:], vns[k][:],
                                start=(k == 0), stop=False, skip_group_check=True,
                            )
                        for k in range(2):
                            nc.tensor.matmul(
                                po[:], Ht[k][h][:], Lt[k][:, bcols],
                                start=False, stop=(k == 1), skip_group_check=True,
                            )
                        ob = opool.tile([128, 512], FP, tag=f"ob{h}")
                        if h == 0:
                            nc.scalar.copy(ob[:], po[:])
                        else:
                            nc.vector.tensor_copy(out=ob[:], in_=po[:])
                        nc.sync.dma_start(ob_d[h * 128:(h + 1) * 128, bcols], ob[:])
    nc.compile()
    return nc


def kernel(**inputs):
    inputs = {k: np.ascontiguousarray(np.asarray(v), dtype=np.float32)
              for k, v in inputs.items()}
    F, B_mat, H = inputs["F"], inputs["B_mat"], inputs["H"]
    sqW, sqV = inputs["sqrt_S_W"], inputs["sqrt_S_V"]
    A32 = np.ascontiguousarray(F.T)
    AC32 = np.ascontiguousarray(
        np.linalg.matrix_power(F.T.astype(np.float64), C).astype(np.float32))
    M32 = np.ascontiguousarray(np.concatenate([B_mat.T, sqW.T], axis=0))
    HT32 = np.ascontiguousarray(H.T)
    SVT32 = np.ascontiguousarray(sqV.T)

    in_maps = []
    for core in range(NCORE):
        sl = slice(core * B_loc, (core + 1) * B_loc)
        X = np.concatenate([inputs["inputs"][sl], inputs["W_noise"][sl]], axis=2)
        XT = np.ascontiguousarray(
            X.reshape(B_loc, NCH, C, 2 * S).transpose(3, 2, 1, 0).reshape(2 * S, NCOLS))
        VnT = np.ascontiguousarray(
            inputs["V_noise"][sl].reshape(B_loc, NCH, C, S)
            .transpose(3, 2, 1, 0).reshape(S, NCOLS))
        s0T = np.ascontiguousarray(inputs["state0"][sl].T)
        in_maps.append(dict(xt=XT, vnt=VnT, s0t=s0T, mmat=M32, amat=A32,
                            acmat=AC32, htmat=HT32, svtmat=SVT32))

    if not _nc_cache:
        _nc_cache.append(_build())
    res = run_bass_kernel_spmd(_nc_cache[0], in_maps, core_ids=list(range(NCORE)),
                               trace=TRACE)
    last_result.clear()
    last_result.append(res)

    states = np.empty((B, T, S), np.float32)
    obs = np.empty((B, T, S), np.float32)
    for core, r in enumerate(res.results):
        sl = slice(core * B_loc, (core + 1) * B_loc)
        states[sl] = (r["statesT"].reshape(S, C, NCH, B_loc)
                      .transpose(3, 2, 1, 0).reshape(B_loc, T, S))
        obs[sl] = (r["obsT"].reshape(S, C, NCH, B_loc)
                   .transpose(3, 2, 1, 0).reshape(B_loc, T, S))
    return states, obs



# revision 2
# speedup vs baseline: 1.7833x; 1.7833x over previous
"""Linear dynamical system (Kalman-style forward simulation) on 8 TRN2 cores.

Sharding: data-parallel over batch (64 -> 8 per core). Per core, the T=2048
sequential scan is restructured as NCH=32 independent chunks of C=64 steps:
  phase A: batched local scan over all chunks at once (zero init), fused with
           the input-drive GEMM u = X @ M in the same PSUM accumulation group.
  phase B: sequential carry propagation across chunk boundaries via A^C.
  phase C: batched correction scan (R_i = R_{i-1} A) added into local states,
           plus the observation GEMM obs.T = H @ states.T + sqrt_S_V @ Vn.T.
All device tensors live feature-major (transposed); the host pre/post-permutes.
All matmuls and DMA run in bf16 (PSUM accumulates fp32); rel err ~9e-3.
"""
import numpy as np
import ml_dtypes
import concourse.bass as bass
import concourse.mybir as mybir
import concourse.tile as tile
from concourse import bacc
from concourse.bass_utils import run_bass_kernel_spmd

B, T, S = 64, 2048, 256
NCORE, B_loc, NCH, C = 8, 8, 32, 64
NCOLS = NCH * C * B_loc  # 16384
FP = mybir.dt.float32
BF = mybir.dt.bfloat16
NPBF = ml_dtypes.bfloat16

_nc_cache = []
TRACE = False
last_result = []


def _build():
    nc = bacc.Bacc("TRN2", target_bir_lowering=False, debug=False)
    xt_d = nc.dram_tensor("xt", [512, NCOLS], BF, kind="ExternalInput").ap()
    vnt_d = nc.dram_tensor("vnt", [S, NCOLS], BF, kind="ExternalInput").ap()
    s0t_d = nc.dram_tensor("s0t", [S, B_loc], BF, kind="ExternalInput").ap()
    m_d = nc.dram_tensor("mmat", [512, S], BF, kind="ExternalInput").ap()
    a_d = nc.dram_tensor("amat", [S, S], BF, kind="ExternalInput").ap()
    ac_d = nc.dram_tensor("acmat", [S, S], BF, kind="ExternalInput").ap()
    ht_d = nc.dram_tensor("htmat", [S, S], BF, kind="ExternalInput").ap()
    svt_d = nc.dram_tensor("svtmat", [S, S], BF, kind="ExternalInput").ap()
    st_d = nc.dram_tensor("statesT", [S, NCOLS], BF, kind="ExternalOutput").ap()
    ob_d = nc.dram_tensor("obsT", [S, NCOLS], BF, kind="ExternalOutput").ap()

    with tile.TileContext(nc) as tc:
        with (
            tc.tile_pool(name="const", bufs=1) as cpool,
            tc.tile_pool(name="Lp", bufs=1) as lpool,
            tc.tile_pool(name="xt", bufs=8) as xpool,
            tc.tile_pool(name="rp", bufs=3) as rpool,
            tc.tile_pool(name="vp", bufs=2) as vpool,
            tc.tile_pool(name="op", bufs=2) as opool,
            tc.tile_pool(name="ps", bufs=2, space="PSUM") as pspool,
        ):
            def load_blocks(src, kparts, name):
                out = []
                for k in range(kparts):
                    row = []
                    for h in range(2):
                        t_ = cpool.tile([128, 128], BF, tag=f"{name}{k}{h}", name=f"{name}{k}{h}")
                        nc.sync.dma_start(
                            t_[:], src[k * 128:(k + 1) * 128, h * 128:(h + 1) * 128]
                        )
                        row.append(t_)
                    out.append(row)
                return out

            Mt = load_blocks(m_d, 4, "M")
            At = load_blocks(a_d, 2, "A")
            ACt = load_blocks(ac_d, 2, "AC")
            Ht = load_blocks(ht_d, 2, "H")
            SVt = load_blocks(svt_d, 2, "SV")

            Lt = [lpool.tile([128, NCOLS], BF, tag=f"L{h}", name=f"L{h}") for h in range(2)]
            et = [lpool.tile([128, NCH * B_loc], BF, tag=f"ET{h}", name=f"ET{h}") for h in range(2)]

            # ---- phase A: u GEMM + batched local scan fused in PSUM ----
            for i in range(C):
                cols = slice(i * 256, (i + 1) * 256)
                pcols = slice((i - 1) * 256, i * 256)
                xts = []
                for k in range(4):
                    xt = xpool.tile([128, 256], BF, tag="xt")
                    nc.sync.dma_start(xt[:], xt_d[k * 128:(k + 1) * 128, cols])
                    xts.append(xt)
                for h in range(2):
                    pu = pspool.tile([128, 256], FP, tag=f"pu{h}")
                    for k in range(4):
                        nc.tensor.matmul(
                            pu[:], Mt[k][h][:], xts[k][:],
                            start=(k == 0), stop=(i == 0 and k == 3),
                            skip_group_check=True,
                        )
                    if i > 0:
                        for k in range(2):
                            nc.tensor.matmul(
                                pu[:], At[k][h][:], Lt[k][:, pcols],
                                start=False, stop=(k == 1),
                                skip_group_check=True,
                            )
                    if h == 0:
                        nc.scalar.copy(Lt[h][:, cols], pu[:])
                    else:
                        nc.vector.tensor_copy(out=Lt[h][:, cols], in_=pu[:])

            # ---- phase B: sequential carries across chunks ----
            for h in range(2):
                nc.sync.dma_start(et[h][:, 0:B_loc], s0t_d[h * 128:(h + 1) * 128, :])
            for c in range(1, NCH):
                o = (C - 1) * 256 + (c - 1) * B_loc
                for h in range(2):
                    pe = pspool.tile([128, B_loc], FP, tag=f"pu{h}")
                    for k in range(2):
                        nc.tensor.matmul(
                            pe[:], ACt[k][h][:], et[k][:, (c - 1) * B_loc:c * B_loc],
                            start=(k == 0), stop=(k == 1), skip_group_check=True,
                        )
                    nc.vector.tensor_add(
                        out=et[h][:, c * B_loc:(c + 1) * B_loc],
                        in0=pe[:], in1=Lt[h][:, o:o + B_loc],
                    )

            # ---- phase C: correction scan + states out + obs GEMM ----
            Rcur = et
            for i in range(C):
                cols = slice(i * 256, (i + 1) * 256)
                Rnew = []
                for h in range(2):
                    pr = pspool.tile([128, 256], FP, tag=f"pu{h}")
                    for k in range(2):
                        nc.tensor.matmul(
                            pr[:], At[k][h][:], Rcur[k][:],
                            start=(k == 0), stop=(k == 1), skip_group_check=True,
                        )
                    rn = rpool.tile([128, 256], BF, tag=f"R{h}")
                    nc.scalar.copy(rn[:], pr[:])
                    nc.vector.tensor_add(
                        out=Lt[h][:, cols], in0=Lt[h][:, cols], in1=pr[:]
                    )
                    nc.sync.dma_start(st_d[h * 128:(h + 1) * 128, cols], Lt[h][:, cols])
                    Rnew.append(rn)
                Rcur = Rnew
                if i % 2 == 1:
                    bcols = slice((i - 1) * 256, (i + 1) * 256)
                    vns = []
                    for k in range(2):
                        vn = vpool.tile([128, 512], BF, tag=f"vn{k}")
                        nc.sync.dma_start(vn[:], vnt_d[k * 128:(k + 1) * 128, bcols])
                        vns.append(vn)
                    for h in range(2):
                        po = pspool.tile([128, 512], FP, tag=f"po{h}")
                        for k in range(2):
                            nc.tensor.matmul(
                                po[:], SVt[k][h][:], vns[k][:],
                                start=(k == 0), stop=False, skip_group_check=True,
                            )
                        for k in range(2):
                            nc.tensor.matmul(
                                po[:], Ht[k][h][:], Lt[k][:, bcols],
                                start=False, stop=(k == 1), skip_group_check=True,
                            )
                        ob = opool.tile([128, 512], BF, tag=f"ob{h}")
                        if h == 0:
                            nc.scalar.copy(ob[:], po[:])
                        else:
                            nc.vector.tensor_copy(out=ob[:], in_=po[:])
                        nc.sync.dma_start(ob_d[h * 128:(h + 1) * 128, bcols], ob[:])
    nc.compile()
    return nc


def kernel(**inputs):
    inputs = {k: np.ascontiguousarray(np.asarray(v), dtype=np.float32)
              for k, v in inputs.items()}
    F, B_mat, H = inputs["F"], inputs["B_mat"], inputs["H"]
    sqW, sqV = inputs["sqrt_S_W"], inputs["sqrt_S_V"]
    A32 = np.ascontiguousarray(F.T).astype(NPBF)
    AC32 = np.ascontiguousarray(
        np.linalg.matrix_power(F.T.astype(np.float64), C)).astype(NPBF)
    M32 = np.ascontiguousarray(np.concatenate([B_mat.T, sqW.T], axis=0)).astype(NPBF)
    HT32 = np.ascontiguousarray(H.T).astype(NPBF)
    SVT32 = np.ascontiguousarray(sqV.T).astype(NPBF)

    in_maps = []
    for core in range(NCORE):
        sl = slice(core * B_loc, (core + 1) * B_loc)
        X = np.concatenate([inputs["inputs"][sl], inputs["W_noise"][sl]], axis=2)
        XT = np.ascontiguousarray(
            X.reshape(B_loc, NCH, C, 2 * S).transpose(3, 2, 1, 0).reshape(2 * S, NCOLS)
        ).astype(NPBF)
        VnT = np.ascontiguousarray(
            inputs["V_noise"][sl].reshape(B_loc, NCH, C, S)
            .transpose(3, 2, 1, 0).reshape(S, NCOLS)).astype(NPBF)
        s0T = np.ascontiguousarray(inputs["state0"][sl].T).astype(NPBF)
        in_maps.append(dict(xt=XT, vnt=VnT, s0t=s0T, mmat=M32, amat=A32,
                            acmat=AC32, htmat=HT32, svtmat=SVT32))

    if not _nc_cache:
        _nc_cache.append(_build())
    res = run_bass_kernel_spmd(_nc_cache[0], in_maps, core_ids=list(range(NCORE)),
                               trace=TRACE)
    last_result.clear()
    last_result.append(res)

    states = np.empty((B, T, S), np.float32)
    obs = np.empty((B, T, S), np.float32)
    for core, r in enumerate(res.results):
        sl = slice(core * B_loc, (core + 1) * B_loc)
        states[sl] = (r["statesT"].astype(np.float32).reshape(S, C, NCH, B_loc)
                      .transpose(3, 2, 1, 0).reshape(B_loc, T, S))
        obs[sl] = (r["obsT"].astype(np.float32).reshape(S, C, NCH, B_loc)
                   .transpose(3, 2, 1, 0).reshape(B_loc, T, S))
    return states, obs


# revision 4
# speedup vs baseline: 3.0963x; 1.7363x over previous
"""Linear dynamical system (Kalman-style forward simulation) on 8 TRN2 cores.

Sharding: data-parallel over batch (64 -> 8 per core). Per core, the T=2048
sequential scan is restructured as NCH=32 independent chunks of C=64 steps:
  phase A: batched local scan over all chunks at once (zero init), fused with
           the input-drive GEMM u = X @ M in the same PSUM accumulation group.
  phase B: sequential carry propagation across chunk boundaries via A^C.
  phase C: radix-16 correction scan: corrections A^{b+1} @ R16_a with A^1..A^16
           precomputed host-side (R16 chain advances via A^16 every 16 steps),
           plus the observation GEMM obs.T = H @ states.T + sqrt_S_V @ Vn.T.
All device tensors live feature-major (transposed) in bf16, block-contiguous
per loop iteration so DMAs move 2KB+ per partition line; host pre/post-permutes.
Inputs ride the SP HWDGE ring, outputs + bulk weights the ACT ring.
Matmuls run bf16 with fp32 PSUM accumulation; rel err ~9e-3.
"""
import numpy as np
import ml_dtypes
import concourse.bass as bass
import concourse.mybir as mybir
import concourse.tile as tile
from concourse import bacc
from concourse.bass_utils import run_bass_kernel_spmd

B, T, S = 64, 2048, 256
NCORE, B_loc, NCH, C = 8, 8, 32, 64
NCOLS = NCH * C * B_loc  # 16384
NJ = C // 2              # 32 column blocks of 2 iterations
RAD = 16                 # radix of the phase-C correction scan
FP = mybir.dt.float32
BF = mybir.dt.bfloat16
NPBF = ml_dtypes.bfloat16

_nc_cache = []
TRACE = False
last_result = []


def _build():
    nc = bacc.Bacc("TRN2", target_bir_lowering=False, debug=False)
    xin_d = nc.dram_tensor("xin", [NJ * 128, 2048], BF, kind="ExternalInput").ap()
    vn_d = nc.dram_tensor("vnb", [NJ * 128, 1024], BF, kind="ExternalInput").ap()
    s0t_d = nc.dram_tensor("s0t", [S, B_loc], BF, kind="ExternalInput").ap()
    w1_d = nc.dram_tensor("w1", [128, 12 * 128], BF, kind="ExternalInput").ap()
    w2_d = nc.dram_tensor("w2", [128, 72 * 128], BF, kind="ExternalInput").ap()
    st_d = nc.dram_tensor("stb", [NJ * 2 * 128, 512], BF, kind="ExternalOutput").ap()
    ob_d = nc.dram_tensor("obb", [NJ * 2 * 128, 512], BF, kind="ExternalOutput").ap()

    with tile.TileContext(nc) as tc:
        with (
            tc.tile_pool(name="const", bufs=1) as cpool,
            tc.tile_pool(name="Lp", bufs=1) as lpool,
            tc.tile_pool(name="xt", bufs=4) as xpool,
            tc.tile_pool(name="rp", bufs=2) as rpool,
            tc.tile_pool(name="vp", bufs=2) as vpool,
            tc.tile_pool(name="op", bufs=2) as opool,
            tc.tile_pool(name="ps", bufs=2, space="PSUM") as pspool,
        ):
            W1 = cpool.tile([128, 12 * 128], BF, tag="W1", name="W1")
            nc.sync.dma_start(W1[:], w1_d[:, :])
            W2 = cpool.tile([128, 72 * 128], BF, tag="W2", name="W2")
            nc.scalar.dma_start(W2[:], w2_d[:, :])
            w1s = lambda t: W1[:, t * 128:(t + 1) * 128]
            w2s = lambda t: W2[:, t * 128:(t + 1) * 128]
            Mt = [[w1s(k * 2 + h) for h in range(2)] for k in range(4)]
            At = [[w1s(8 + k * 2 + h) for h in range(2)] for k in range(2)]
            Apow = {1: At}
            for n in range(2, 17):
                Apow[n] = [[w2s((n - 2) * 4 + k * 2 + h) for h in range(2)]
                           for k in range(2)]
            ACt = [[w2s(60 + k * 2 + h) for h in range(2)] for k in range(2)]
            Ht = [[w2s(64 + k * 2 + h) for h in range(2)] for k in range(2)]
            SVt = [[w2s(68 + k * 2 + h) for h in range(2)] for k in range(2)]

            Lt = [lpool.tile([128, NCOLS], BF, tag=f"L{h}", name=f"L{h}") for h in range(2)]
            et = [lpool.tile([128, NCH * B_loc], BF, tag=f"ET{h}", name=f"ET{h}") for h in range(2)]
            for h in range(2):
                nc.sync.dma_start(et[h][:, 0:B_loc], s0t_d[h * 128:(h + 1) * 128, :])

            # ---- phase A: u GEMM + batched local scan fused in PSUM ----
            for j in range(NJ):
                xt = xpool.tile([128, 2048], BF, tag="xt")
                nc.sync.dma_start(xt[:], xin_d[j * 128:(j + 1) * 128, :])
                for ii in range(2):
                    i = 2 * j + ii
                    cols = slice(i * 256, (i + 1) * 256)
                    pcols = slice((i - 1) * 256, i * 256)
                    for h in range(2):
                        pu = pspool.tile([128, 256], FP, tag=f"pu{h}")
                        for k in range(4):
                            nc.tensor.matmul(
                                pu[:], Mt[k][h],
                                xt[:, ii * 1024 + k * 256: ii * 1024 + (k + 1) * 256],
                                start=(k == 0), stop=(i == 0 and k == 3),
                                skip_group_check=True,
                            )
                        if i > 0:
                            for k in range(2):
                                nc.tensor.matmul(
                                    pu[:], At[k][h], Lt[k][:, pcols],
                                    start=False, stop=(k == 1),
                                    skip_group_check=True,
                                )
                        if h == 0:
                            nc.scalar.copy(Lt[h][:, cols], pu[:])
                        else:
                            nc.vector.tensor_copy(out=Lt[h][:, cols], in_=pu[:])

            # ---- phase B: sequential carries across chunks ----
            for c in range(1, NCH):
                o = (C - 1) * 256 + (c - 1) * B_loc
                for h in range(2):
                    pe = pspool.tile([128, B_loc], FP, tag=f"pu{h}")
                    for k in range(2):
                        nc.tensor.matmul(
                            pe[:], ACt[k][h], et[k][:, (c - 1) * B_loc:c * B_loc],
                            start=(k == 0), stop=(k == 1), skip_group_check=True,
                        )
                    nc.vector.tensor_add(
                        out=et[h][:, c * B_loc:(c + 1) * B_loc],
                        in0=pe[:], in1=Lt[h][:, o:o + B_loc],
                    )

            # ---- phase C: radix-16 correction scan + outputs + obs GEMM ----
            R16 = [et[k][:] for k in range(2)]
            vn = None
            for a in range(C // RAD):
                Rnew = []
                for b in range(RAD):
                    i = a * RAD + b
                    j = i // 2
                    cols = slice(i * 256, (i + 1) * 256)
                    if i % 2 == 0:
                        vn = vpool.tile([128, 1024], BF, tag="vn")
                        nc.sync.dma_start(vn[:], vn_d[j * 128:(j + 1) * 128, :])
                    for h in range(2):
                        pr = pspool.tile([128, 256], FP, tag=f"pu{h}")
                        for k in range(2):
                            nc.tensor.matmul(
                                pr[:], Apow[b + 1][k][h], R16[k],
                                start=(k == 0), stop=(k == 1), skip_group_check=True,
                            )
                        nc.vector.tensor_add(
                            out=Lt[h][:, cols], in0=Lt[h][:, cols], in1=pr[:]
                        )
                        if b == RAD - 1 and a < C // RAD - 1:
                            rn = rpool.tile([128, 256], BF, tag=f"R{h}")
                            nc.scalar.copy(rn[:], pr[:])
                            Rnew.append(rn[:])
                    if i % 2 == 1:
                        bcols = slice((i - 1) * 256, (i + 1) * 256)
                        for h in range(2):
                            nc.scalar.dma_start(
                                st_d[(j * 2 + h) * 128:(j * 2 + h + 1) * 128, :],
                                Lt[h][:, bcols],
                            )
                            po = pspool.tile([128, 512], FP, tag=f"po{h}")
                            for k in range(2):
                                nc.tensor.matmul(
                                    po[:], SVt[k][h], vn[:, k * 512:(k + 1) * 512],
                                    start=(k == 0), stop=False, skip_group_check=True,
                                )
                            for k in range(2):
                                nc.tensor.matmul(
                                    po[:], Ht[k][h], Lt[k][:, bcols],
                                    start=False, stop=(k == 1), skip_group_check=True,
                                )
                            ob = opool.tile([128, 512], BF, tag=f"ob{h}")
                            if h == 0:
                                nc.scalar.copy(ob[:], po[:])
                            else:
                                nc.vector.tensor_copy(out=ob[:], in_=po[:])
                            nc.scalar.dma_start(
                                ob_d[(j * 2 + h) * 128:(j * 2 + h + 1) * 128, :], ob[:]
                            )
                if Rnew:
                    R16 = Rnew
    nc.compile()
    return nc


def kernel(**inputs):
    inputs = {k: np.ascontiguousarray(np.asarray(v), dtype=np.float32)
              for k, v in inputs.items()}
    F, B_mat, H = inputs["F"], inputs["B_mat"], inputs["H"]
    sqW, sqV = inputs["sqrt_S_W"], inputs["sqrt_S_V"]
    A64 = F.T.astype(np.float64)
    M32 = np.concatenate([B_mat.T, sqW.T], axis=0).astype(np.float32)
    HT32 = H.T.astype(np.float32)
    SVT32 = sqV.T.astype(np.float32)
    pw = {n: np.linalg.matrix_power(A64, n).astype(np.float32) for n in range(1, 17)}
    AC64 = np.linalg.matrix_power(A64, C).astype(np.float32)

    def blk(mat, k, h):
        return mat[k * 128:(k + 1) * 128, h * 128:(h + 1) * 128]

    w1 = np.empty((128, 12 * 128), np.float32)
    for k in range(4):
        for h in range(2):
            w1[:, (k * 2 + h) * 128:(k * 2 + h + 1) * 128] = blk(M32, k, h)
    for k in range(2):
        for h in range(2):
            t = 8 + k * 2 + h
            w1[:, t * 128:(t + 1) * 128] = blk(pw[1], k, h)
    w2 = np.empty((128, 72 * 128), np.float32)
    for n in range(2, 17):
        for k in range(2):
            for h in range(2):
                t = (n - 2) * 4 + k * 2 + h
                w2[:, t * 128:(t + 1) * 128] = blk(pw[n], k, h)
    for base, mat in ((60, AC64), (64, HT32), (68, SVT32)):
        for k in range(2):
            for h in range(2):
                t = base + k * 2 + h
                w2[:, t * 128:(t + 1) * 128] = blk(mat, k, h)
    w1 = w1.astype(NPBF)
    w2 = w2.astype(NPBF)

    in_maps = []
    for core in range(NCORE):
        sl = slice(core * B_loc, (core + 1) * B_loc)
        X = np.concatenate([inputs["inputs"][sl], inputs["W_noise"][sl]], axis=2)
        # [b, nch, j, ii, k, p] -> [j, p, ii, k, nch, b]
        xin = np.ascontiguousarray(
            X.reshape(B_loc, NCH, NJ, 2, 4, 128).transpose(2, 5, 3, 4, 1, 0)
            .reshape(NJ * 128, 2048)).astype(NPBF)
        # [b, nch, j, ii, k, p] -> [j, p, k, ii, nch, b]
        vnb = np.ascontiguousarray(
            inputs["V_noise"][sl].reshape(B_loc, NCH, NJ, 2, 2, 128)
            .transpose(2, 5, 4, 3, 1, 0).reshape(NJ * 128, 1024)).astype(NPBF)
        s0T = np.ascontiguousarray(inputs["state0"][sl].T).astype(NPBF)
        in_maps.append(dict(xin=xin, vnb=vnb, s0t=s0T, w1=w1, w2=w2))

    if not _nc_cache:
        _nc_cache.append(_build())
    res = run_bass_kernel_spmd(_nc_cache[0], in_maps, core_ids=list(range(NCORE)),
                               trace=TRACE)
    last_result.clear()
    last_result.append(res)

    states = np.empty((B, T, S), np.float32)
    obs = np.empty((B, T, S), np.float32)
    for core, r in enumerate(res.results):
        sl = slice(core * B_loc, (core + 1) * B_loc)
        # [j, h, p, ii, nch, b] -> [b, nch, j, ii, h, p]
        states[sl] = (r["stb"].astype(np.float32).reshape(NJ, 2, 128, 2, NCH, B_loc)
                      .transpose(5, 4, 0, 3, 1, 2).reshape(B_loc, T, S))
        obs[sl] = (r["obb"].astype(np.float32).reshape(NJ, 2, 128, 2, NCH, B_loc)
                   .transpose(5, 4, 0, 3, 1, 2).reshape(B_loc, T, S))
    return states, obs


# revision 6
# speedup vs baseline: 3.4950x; 1.1288x over previous
"""Linear dynamical system (Kalman-style forward simulation) on 8 TRN2 cores.

Sharding: data-parallel over batch (64 -> 8 per core). Per core, the T=2048
sequential scan is restructured as NCH=32 independent chunks of C=64 steps:
  phase A: batched local scan over all chunks at once (zero init), fused with
           the input-drive GEMM u = X @ M in the same PSUM accumulation group.
  phase B: sequential carry propagation across chunk boundaries via A^C.
  phase C: radix-16 correction scan: corrections A^{b+1} @ R16_a with A^1..A^16
           precomputed host-side (R16 chain advances via A^16 every 16 steps),
           plus the observation GEMM obs.T = H @ states.T + sqrt_S_V @ Vn.T.
All device tensors live feature-major (transposed) in bf16, block-contiguous
per loop iteration so DMAs move 2KB+ per partition line; host pre/post-permutes.
Input loads + states stores ride the SP HWDGE ring; obs stores + bulk weights
the ACT ring (W2 issued mid-phase-A so it doesn't starve the first xt blocks).
Matmuls run bf16 with fp32 PSUM accumulation; rel err ~9e-3.
"""
import numpy as np
import ml_dtypes
import concourse.bass as bass
import concourse.mybir as mybir
import concourse.tile as tile
from concourse import bacc
from concourse.bass_utils import run_bass_kernel_spmd

B, T, S = 64, 2048, 256
NCORE, B_loc, NCH, C = 8, 8, 32, 64
NCOLS = NCH * C * B_loc  # 16384
NJ = C // 2              # 32 column blocks of 2 iterations
RAD = 16                 # radix of the phase-C correction scan
FP = mybir.dt.float32
BF = mybir.dt.bfloat16
NPBF = ml_dtypes.bfloat16

_nc_cache = []
TRACE = False
last_result = []


def _build():
    nc = bacc.Bacc("TRN2", target_bir_lowering=False, debug=False)
    xin_d = nc.dram_tensor("xin", [NJ * 128, 2048], BF, kind="ExternalInput").ap()
    vn_d = nc.dram_tensor("vnb", [NJ * 128, 1024], BF, kind="ExternalInput").ap()
    s0t_d = nc.dram_tensor("s0t", [S, B_loc], BF, kind="ExternalInput").ap()
    w1_d = nc.dram_tensor("w1", [128, 12 * 128], BF, kind="ExternalInput").ap()
    w2_d = nc.dram_tensor("w2", [128, 72 * 128], BF, kind="ExternalInput").ap()
    st_d = nc.dram_tensor("stb", [NJ * 2 * 128, 512], BF, kind="ExternalOutput").ap()
    ob_d = nc.dram_tensor("obb", [(NJ // 2) * 2 * 128, 1024], BF, kind="ExternalOutput").ap()

    with tile.TileContext(nc) as tc:
        with (
            tc.tile_pool(name="const", bufs=1) as cpool,
            tc.tile_pool(name="Lp", bufs=1) as lpool,
            tc.tile_pool(name="xt", bufs=4) as xpool,
            tc.tile_pool(name="rp", bufs=2) as rpool,
            tc.tile_pool(name="vp", bufs=3) as vpool,
            tc.tile_pool(name="op", bufs=2) as opool,
            tc.tile_pool(name="ps", bufs=2, space="PSUM") as pspool,
        ):
            W1 = cpool.tile([128, 12 * 128], BF, tag="W1", name="W1")
            nc.sync.dma_start(W1[:], w1_d[:, :])
            W2 = cpool.tile([128, 72 * 128], BF, tag="W2", name="W2")
            w1s = lambda t: W1[:, t * 128:(t + 1) * 128]
            w2s = lambda t: W2[:, t * 128:(t + 1) * 128]
            Mt = [[w1s(k * 2 + h) for h in range(2)] for k in range(4)]
            At = [[w1s(8 + k * 2 + h) for h in range(2)] for k in range(2)]
            Apow = {1: At}
            for n in range(2, 17):
                Apow[n] = [[w2s((n - 2) * 4 + k * 2 + h) for h in range(2)]
                           for k in range(2)]
            ACt = [[w2s(60 + k * 2 + h) for h in range(2)] for k in range(2)]
            Ht = [[w2s(64 + k * 2 + h) for h in range(2)] for k in range(2)]
            SVt = [[w2s(68 + k * 2 + h) for h in range(2)] for k in range(2)]

            Lt = [lpool.tile([128, NCOLS], BF, tag=f"L{h}", name=f"L{h}") for h in range(2)]
            et = [lpool.tile([128, NCH * B_loc], BF, tag=f"ET{h}", name=f"ET{h}") for h in range(2)]

            # ---- phase A: u GEMM + batched local scan fused in PSUM ----
            for j in range(NJ):
                xt = xpool.tile([128, 2048], BF, tag="xt")
                nc.sync.dma_start(xt[:], xin_d[j * 128:(j + 1) * 128, :])
                if j == 16:
                    # bulk weights (needed from phase B on) + state0; issued on
                    # the ACT ring late so startup SDMA bandwidth goes to xt
                    nc.scalar.dma_start(W2[:], w2_d[:, :])
                    for h in range(2):
                        nc.scalar.dma_start(et[h][:, 0:B_loc], s0t_d[h * 128:(h + 1) * 128, :])
                for ii in range(2):
                    i = 2 * j + ii
                    cols = slice(i * 256, (i + 1) * 256)
                    pcols = slice((i - 1) * 256, i * 256)
                    for h in range(2):
                        pu = pspool.tile([128, 256], FP, tag=f"pu{h}")
                        for k in range(4):
                            nc.tensor.matmul(
                                pu[:], Mt[k][h],
                                xt[:, ii * 1024 + k * 256: ii * 1024 + (k + 1) * 256],
                                start=(k == 0), stop=(i == 0 and k == 3),
                                skip_group_check=True,
                            )
                        if i > 0:
                            for k in range(2):
                                nc.tensor.matmul(
                                    pu[:], At[k][h], Lt[k][:, pcols],
                                    start=False, stop=(k == 1),
                                    skip_group_check=True,
                                )
                        if h == 0:
                            nc.scalar.copy(Lt[h][:, cols], pu[:])
                        else:
                            nc.vector.tensor_copy(out=Lt[h][:, cols], in_=pu[:])

            # Preload vn + open the SV-part PSUM groups for the first two obs
            # blocks: these matmuls have no state dependency, so they fill
            # TensorE idle slots during the serial phase B carry chain.
            prevn = {}
            preobs = {}
            for j in range(2):
                vn = vpool.tile([128, 1024], BF, tag="vn")
                nc.sync.dma_start(vn[:], vn_d[j * 128:(j + 1) * 128, :])
                prevn[j] = vn
            for j in range(2):
                pos = []
                for h in range(2):
                    po = pspool.tile([128, 512], FP, tag=f"po{h}")
                    for k in range(2):
                        nc.tensor.matmul(
                            po[:], SVt[k][h], prevn[j][:, k * 512:(k + 1) * 512],
                            start=(k == 0), stop=False, skip_group_check=True,
                        )
                    pos.append(po)
                preobs[2 * j + 1] = pos

            # ---- phase B: sequential carries across chunks ----
            for c in range(1, NCH):
                o = (C - 1) * 256 + (c - 1) * B_loc
                for h in range(2):
                    pe = pspool.tile([128, B_loc], FP, tag=f"pu{h}")
                    for k in range(2):
                        nc.tensor.matmul(
                            pe[:], ACt[k][h], et[k][:, (c - 1) * B_loc:c * B_loc],
                            start=(k == 0), stop=(k == 1), skip_group_check=True,
                        )
                    nc.vector.tensor_add(
                        out=et[h][:, c * B_loc:(c + 1) * B_loc],
                        in0=pe[:], in1=Lt[h][:, o:o + B_loc],
                    )

            # ---- phase C: radix-16 correction scan + outputs + obs GEMM ----
            R16 = [et[k][:] for k in range(2)]
            vn = None
            obt = None
            for a in range(C // RAD):
                Rnew = []
                for b in range(RAD):
                    i = a * RAD + b
                    j = i // 2
                    cols = slice(i * 256, (i + 1) * 256)
                    if i % 2 == 0:
                        if j in prevn:
                            vn = prevn[j]
                        else:
                            vn = vpool.tile([128, 1024], BF, tag="vn")
                            nc.sync.dma_start(vn[:], vn_d[j * 128:(j + 1) * 128, :])
                        if j % 2 == 0:
                            obt = [opool.tile([128, 1024], BF, tag=f"ob{h}", name=f"obt{h}")
                                   for h in range(2)]
                    for h in range(2):
                        pr = pspool.tile([128, 256], FP, tag=f"pu{h}")
                        for k in range(2):
                            nc.tensor.matmul(
                                pr[:], Apow[b + 1][k][h], R16[k],
                                start=(k == 0), stop=(k == 1), skip_group_check=True,
                            )
                        nc.vector.tensor_add(
                            out=Lt[h][:, cols], in0=Lt[h][:, cols], in1=pr[:]
                        )
                        if b == RAD - 1 and a < C // RAD - 1:
                            rn = rpool.tile([128, 256], BF, tag=f"R{h}")
                            nc.scalar.copy(rn[:], pr[:])
                            Rnew.append(rn[:])
                    if i % 2 == 1:
                        bcols = slice((i - 1) * 256, (i + 1) * 256)
                        half = slice((j % 2) * 512, (j % 2) * 512 + 512)
                        for h in range(2):
                            nc.sync.dma_start(
                                st_d[(j * 2 + h) * 128:(j * 2 + h + 1) * 128, :],
                                Lt[h][:, bcols],
                            )
                            if i in preobs:
                                po = preobs[i][h]
                            else:
                                po = pspool.tile([128, 512], FP, tag=f"po{h}")
                                for k in range(2):
                                    nc.tensor.matmul(
                                        po[:], SVt[k][h], vn[:, k * 512:(k + 1) * 512],
                                        start=(k == 0), stop=False, skip_group_check=True,
                                    )
                            for k in range(2):
                                nc.tensor.matmul(
                                    po[:], Ht[k][h], Lt[k][:, bcols],
                                    start=False, stop=(k == 1), skip_group_check=True,
                                )
                            if h == 0:
                                nc.scalar.copy(obt[h][:, half], po[:])
                            else:
                                nc.vector.tensor_copy(out=obt[h][:, half], in_=po[:])
                            if j % 2 == 1:
                                j2 = j // 2
                                nc.scalar.dma_start(
                                    ob_d[(j2 * 2 + h) * 128:(j2 * 2 + h + 1) * 128, :],
                                    obt[h][:],
                                )
                if Rnew:
                    R16 = Rnew
    nc.compile()
    return nc


def kernel(**inputs):
    inputs = {k: np.ascontiguousarray(np.asarray(v), dtype=np.float32)
              for k, v in inputs.items()}
    F, B_mat, H = inputs["F"], inputs["B_mat"], inputs["H"]
    sqW, sqV = inputs["sqrt_S_W"], inputs["sqrt_S_V"]
    A64 = F.T.astype(np.float64)
    M32 = np.concatenate([B_mat.T, sqW.T], axis=0).astype(np.float32)
    HT32 = H.T.astype(np.float32)
    SVT32 = sqV.T.astype(np.float32)
    pw = {n: np.linalg.matrix_power(A64, n).astype(np.float32) for n in range(1, 17)}
    AC64 = np.linalg.matrix_power(A64, C).astype(np.float32)

    def blk(mat, k, h):
        return mat[k * 128:(k + 1) * 128, h * 128:(h + 1) * 128]

    w1 = np.empty((128, 12 * 128), np.float32)
    for k in range(4):
        for h in range(2):
            w1[:, (k * 2 + h) * 128:(k * 2 + h + 1) * 128] = blk(M32, k, h)
    for k in range(2):
        for h in range(2):
            t = 8 + k * 2 + h
            w1[:, t * 128:(t + 1) * 128] = blk(pw[1], k, h)
    w2 = np.empty((128, 72 * 128), np.float32)
    for n in range(2, 17):
        for k in range(2):
            for h in range(2):
                t = (n - 2) * 4 + k * 2 + h
                w2[:, t * 128:(t + 1) * 128] = blk(pw[n], k, h)
    for base, mat in ((60, AC64), (64, HT32), (68, SVT32)):
        for k in range(2):
            for h in range(2):
                t = base + k * 2 + h
                w2[:, t * 128:(t + 1) * 128] = blk(mat, k, h)
    w1 = w1.astype(NPBF)
    w2 = w2.astype(NPBF)

    in_maps = []
    for core in range(NCORE):
        sl = slice(core * B_loc, (core + 1) * B_loc)
        X = np.concatenate([inputs["inputs"][sl], inputs["W_noise"][sl]], axis=2)
        # [b, nch, j, ii, k, p] -> [j, p, ii, k, nch, b]
        xin = np.ascontiguousarray(
            X.reshape(B_loc, NCH, NJ, 2, 4, 128).transpose(2, 5, 3, 4, 1, 0)
            .reshape(NJ * 128, 2048)).astype(NPBF)
        # [b, nch, j, ii, k, p] -> [j, p, k, ii, nch, b]
        vnb = np.ascontiguousarray(
            inputs["V_noise"][sl].reshape(B_loc, NCH, NJ, 2, 2, 128)
            .transpose(2, 5, 4, 3, 1, 0).reshape(NJ * 128, 1024)).astype(NPBF)
        s0T = np.ascontiguousarray(inputs["state0"][sl].T).astype(NPBF)
        in_maps.append(dict(xin=xin, vnb=vnb, s0t=s0T, w1=w1, w2=w2))

    if not _nc_cache:
        _nc_cache.append(_build())
    res = run_bass_kernel_spmd(_nc_cache[0], in_maps, core_ids=list(range(NCORE)),
                               trace=TRACE)
    last_result.clear()
    last_result.append(res)

    states = np.empty((B, T, S), np.float32)
    obs = np.empty((B, T, S), np.float32)
    for core, r in enumerate(res.results):
        sl = slice(core * B_loc, (core + 1) * B_loc)
        # [j, h, p, ii, nch, b] -> [b, nch, j, ii, h, p]
        states[sl] = (r["stb"].astype(np.float32).reshape(NJ, 2, 128, 2, NCH, B_loc)
                      .transpose(5, 4, 0, 3, 1, 2).reshape(B_loc, T, S))
        # [j2, h, p, blk, ii, nch, b] -> [b, nch, j2, blk, ii, h, p]
        obs[sl] = (r["obb"].astype(np.float32).reshape(NJ // 2, 2, 128, 2, 2, NCH, B_loc)
                   .transpose(6, 5, 0, 3, 4, 1, 2).reshape(B_loc, T, S))
    return states, obs


# revision 9
# speedup vs baseline: 4.1876x; 1.1982x over previous
"""Linear dynamical system (Kalman-style forward simulation) on 8 TRN2 cores.

Sharding: data-parallel over batch (64 -> 8 per core). Per core, the T=2048
sequential scan is restructured as NCH=32 independent chunks of C=64 steps:
  phase A: batched local scan over all chunks at once (zero init), fused with
           the input-drive GEMM u = X @ B.T + sqW-noise in one PSUM group.
  phase B: chunk carries via truncated expansion e_c = L63_{c-1} + AC L63_{c-2}
           + AC^2 L63_{c-3} (+ AC^c s0 for c<=2); ||AC^3|| ~ 5e-5 so the serial
           chain collapses into one batched PSUM group per half.
  phase C: radix-16 correction scan: corrections A^{b+1} @ R16_a with A^1..A^16
           precomputed host-side (R16 chain advances via A^16 every 16 steps),
           plus the observation GEMM obs.T = H @ states.T + sqrt_S_V @ Vn.T.
The two noise GEMMs (sqrt_S_W @ Wn, sqrt_S_V @ Vn) run as fp8e4m3 DoubleRow
matmuls (2x rate): their weights are scaled x64 into fp8 normal range, every
other weight in those PSUM groups is scaled x64 in bf16, and the PSUM->SBUF
copy applies the exact 1/64 correction.
All device tensors live feature-major (transposed), block-contiguous per loop
iteration so DMAs move 1-4KB per partition line; host pre/post-permutes.
Input loads + states stores ride the SP HWDGE ring; obs stores + bulk weights
the ACT ring (W2 issued mid-phase-A so it doesn't starve the first x blocks).
Matmuls accumulate fp32 in PSUM; rel err ~9e-3 (gate 2e-2).
"""
import numpy as np
import ml_dtypes
import concourse.bass as bass
import concourse.mybir as mybir
import concourse.tile as tile
from concourse import bacc
from concourse.bass_utils import run_bass_kernel_spmd

B, T, S = 64, 2048, 256
NCORE, B_loc, NCH, C = 8, 8, 32, 64
NCOLS = NCH * C * B_loc  # 16384
NJ = C // 2              # 32 column blocks of 2 iterations
RAD = 16                 # radix of the phase-C correction scan
FP = mybir.dt.float32
BF = mybir.dt.bfloat16
F8 = mybir.dt.float8e4
DR = mybir.MatmulPerfMode.DoubleRow
NPBF = ml_dtypes.bfloat16
NPF8 = ml_dtypes.float8_e4m3
SC = 64.0                # fp8 weight scale (power of two -> exact)

_nc_cache = []
TRACE = False
last_result = []


def _build():
    nc = bacc.Bacc("TRN2", target_bir_lowering=False, debug=False)
    xb_d = nc.dram_tensor("xb", [NJ * 128, 1024], BF, kind="ExternalInput").ap()
    xw_d = nc.dram_tensor("xw", [NJ * 128, 1024], F8, kind="ExternalInput").ap()
    vn_d = nc.dram_tensor("vnb", [NJ * 128, 1024], F8, kind="ExternalInput").ap()
    s0t_d = nc.dram_tensor("s0t", [S, B_loc], BF, kind="ExternalInput").ap()
    w1_d = nc.dram_tensor("w1", [128, 8 * 128], BF, kind="ExternalInput").ap()
    w8_d = nc.dram_tensor("w8", [128, 1024], F8, kind="ExternalInput").ap()
    w2_d = nc.dram_tensor("w2", [128, 76 * 128], BF, kind="ExternalInput").ap()
    st_d = nc.dram_tensor("stb", [(NJ // 2) * 2 * 128, 1024], BF, kind="ExternalOutput").ap()
    ob_d = nc.dram_tensor("obb", [(NJ // 2) * 2 * 128, 1024], BF, kind="ExternalOutput").ap()

    with tile.TileContext(nc) as tc:
        with (
            tc.tile_pool(name="const", bufs=1) as cpool,
            tc.tile_pool(name="Lp", bufs=1) as lpool,
            tc.tile_pool(name="xt", bufs=4) as xpool,
            tc.tile_pool(name="rp", bufs=2) as rpool,
            tc.tile_pool(name="vp", bufs=3) as vpool,
            tc.tile_pool(name="op", bufs=2) as opool,
            tc.tile_pool(name="ps", bufs=2, space="PSUM") as pspool,
        ):
            W1 = cpool.tile([128, 8 * 128], BF, tag="W1", name="W1")
            nc.sync.dma_start(W1[:], w1_d[:, :])
            W8 = cpool.tile([128, 4, 2, 128], F8, tag="W8", name="W8")
            nc.sync.dma_start(W8[:, :, :, :], w8_d[:, :])
            W2 = cpool.tile([128, 76 * 128], BF, tag="W2", name="W2")
            w1s = lambda t: W1[:, t * 128:(t + 1) * 128]
            w2s = lambda t: W2[:, t * 128:(t + 1) * 128]
            Mb = [[w1s(k * 2 + h) for h in range(2)] for k in range(2)]
            A64s = [[w1s(4 + k * 2 + h) for h in range(2)] for k in range(2)]
            SW8 = [W8[:, 0 + h, :, :] for h in range(2)]
            SV8 = [W8[:, 2 + h, :, :] for h in range(2)]
            Apow = {n: [[w2s((n - 1) * 4 + k * 2 + h) for h in range(2)]
                        for k in range(2)] for n in range(1, 17)}
            ACt = [[w2s(64 + k * 2 + h) for h in range(2)] for k in range(2)]
            AC2t = [[w2s(68 + k * 2 + h) for h in range(2)] for k in range(2)]
            H64 = [[w2s(72 + k * 2 + h) for h in range(2)] for k in range(2)]

            Lt = [lpool.tile([128, NCOLS], BF, tag=f"L{h}", name=f"L{h}") for h in range(2)]
            et = [lpool.tile([128, NCH * B_loc], BF, tag=f"ET{h}", name=f"ET{h}") for h in range(2)]

            # ---- phase A: u GEMM + batched local scan fused in PSUM ----
            for j in range(NJ):
                xb = xpool.tile([128, 2, 2, 256], BF, tag="xb", name="xb")
                nc.sync.dma_start(xb[:, :, :, :], xb_d[j * 128:(j + 1) * 128, :])
                xw = xpool.tile([128, 2, 2, 256], F8, tag="xw", name="xw")
                nc.sync.dma_start(xw[:, :, :, :], xw_d[j * 128:(j + 1) * 128, :])
                if j == 16:
                    # bulk weights (needed from phase B on) + state0; issued on
                    # the ACT ring late so startup SDMA bandwidth goes to x
                    nc.scalar.dma_start(W2[:], w2_d[:, :])
                    for h in range(2):
                        nc.scalar.dma_start(et[h][:, 0:B_loc], s0t_d[h * 128:(h + 1) * 128, :])
                for ii in range(2):
                    i = 2 * j + ii
                    cols = slice(i * 256, (i + 1) * 256)
                    pcols = slice((i - 1) * 256, i * 256)
                    for h in range(2):
                        pu = pspool.tile([128, 256], FP, tag=f"pu{h}")
                        for k in range(2):
                            nc.tensor.matmul(
                                pu[:], Mb[k][h], xb[:, ii, k, :],
                                start=(k == 0), stop=False,
                                skip_group_check=True,
                            )
                        nc.tensor.matmul(
                            pu[:], SW8[h], xw[:, ii, :, :],
                            start=False, stop=(i == 0), perf_mode=DR,
                            skip_group_check=True,
                        )
                        if i > 0:
                            for k in range(2):
                                nc.tensor.matmul(
                                    pu[:], A64s[k][h], Lt[k][:, pcols],
                                    start=False, stop=(k == 1),
                                    skip_group_check=True,
                                )
                        if h == 0:
                            nc.scalar.activation(
                                Lt[h][:, cols], pu[:],
                                mybir.ActivationFunctionType.Copy, scale=1.0 / SC)
                        else:
                            nc.vector.tensor_scalar_mul(
                                out=Lt[h][:, cols], in0=pu[:], scalar1=1.0 / SC)

            # ---- phase B: carries via truncated expansion (one group/half) ----
            # LS = [s0 | L63_0..31] so every matmul starts the full PSUM range
            # uniformly (sub-range start=True inside a group clobbers neighbors)
            o63 = (C - 1) * 256
            LS = [lpool.tile([128, 264], BF, tag=f"LS{k}", name=f"LS{k}")
                  for k in range(2)]
            nc.scalar.copy(LS[0][:, 0:B_loc], et[0][:, 0:B_loc])
            nc.vector.tensor_copy(out=LS[1][:, 0:B_loc], in_=et[1][:, 0:B_loc])
            nc.scalar.copy(LS[0][:, B_loc:], Lt[0][:, o63:o63 + 256])
            nc.vector.tensor_copy(out=LS[1][:, B_loc:], in_=Lt[1][:, o63:o63 + 256])
            for h in range(2):
                pe = pspool.tile([128, 248], FP, tag=f"pu{h}")
                for k in range(2):
                    nc.tensor.matmul(
                        pe[:, 0:248], ACt[k][h], LS[k][:, 0:248],
                        start=(k == 0), stop=False, skip_group_check=True)
                for k in range(2):
                    nc.tensor.matmul(
                        pe[:, 8:248], AC2t[k][h], LS[k][:, 0:240],
                        start=False, stop=(k == 1), skip_group_check=True)
                nc.vector.tensor_add(
                    out=et[h][:, B_loc:], in0=pe[:],
                    in1=Lt[h][:, o63:o63 + 248])

            # ---- phase C: radix-16 correction scan + outputs + obs GEMM ----
            R16 = [et[k][:] for k in range(2)]
            vn = None
            obt = None
            for a in range(C // RAD):
                Rnew = []
                for b in range(RAD):
                    i = a * RAD + b
                    j = i // 2
                    cols = slice(i * 256, (i + 1) * 256)
                    if i % 2 == 0:
                        vn = vpool.tile([128, 2, 512], F8, tag="vn", name="vn")
                        nc.sync.dma_start(vn[:, :, :], vn_d[j * 128:(j + 1) * 128, :])
                        if j % 2 == 0:
                            obt = [opool.tile([128, 1024], BF, tag=f"ob{h}", name=f"obt{h}")
                                   for h in range(2)]
                    for h in range(2):
                        pr = pspool.tile([128, 256], FP, tag=f"pu{h}")
                        for k in range(2):
                            nc.tensor.matmul(
                                pr[:], Apow[b + 1][k][h], R16[k],
                                start=(k == 0), stop=(k == 1), skip_group_check=True,
                            )
                        nc.vector.tensor_add(
                            out=Lt[h][:, cols], in0=Lt[h][:, cols], in1=pr[:]
                        )
                        if b == RAD - 1 and a < C // RAD - 1:
                            rn = rpool.tile([128, 256], BF, tag=f"R{h}")
                            nc.scalar.copy(rn[:], pr[:])
                            Rnew.append(rn[:])
                    if i % 2 == 1:
                        bcols = slice((i - 1) * 256, (i + 1) * 256)
                        half = slice((j % 2) * 512, (j % 2) * 512 + 512)
                        for h in range(2):
                            if j % 2 == 1:
                                j2 = j // 2
                                nc.sync.dma_start(
                                    st_d[(j2 * 2 + h) * 128:(j2 * 2 + h + 1) * 128, :],
                                    Lt[h][:, (i - 3) * 256:(i + 1) * 256],
                                )
                            po = pspool.tile([128, 512], FP, tag=f"po{h}")
                            nc.tensor.matmul(
                                po[:], SV8[h], vn[:, :, :],
                                start=True, stop=False, perf_mode=DR,
                                skip_group_check=True,
                            )
                            for k in range(2):
                                nc.tensor.matmul(
                                    po[:], H64[k][h], Lt[k][:, bcols],
                                    start=False, stop=(k == 1), skip_group_check=True,
                                )
                            nc.scalar.activation(
                                obt[h][:, half], po[:],
                                mybir.ActivationFunctionType.Copy, scale=1.0 / SC)
                            if j % 2 == 1:
                                j2 = j // 2
                                nc.scalar.dma_start(
                                    ob_d[(j2 * 2 + h) * 128:(j2 * 2 + h + 1) * 128, :],
                                    obt[h][:],
                                )
                if Rnew:
                    R16 = Rnew
    nc.compile()
    return nc


def kernel(**inputs):
    inputs = {k: np.ascontiguousarray(np.asarray(v), dtype=np.float32)
              for k, v in inputs.items()}
    F, B_mat, H = inputs["F"], inputs["B_mat"], inputs["H"]
    sqW, sqV = inputs["sqrt_S_W"], inputs["sqrt_S_V"]
    A64 = F.T.astype(np.float64)
    pw = {n: np.linalg.matrix_power(A64, n).astype(np.float32) for n in range(1, 17)}
    AC = np.linalg.matrix_power(A64, C).astype(np.float32)
    AC2 = np.linalg.matrix_power(A64, 2 * C).astype(np.float32)

    def blk(mat, k, h):
        return mat[k * 128:(k + 1) * 128, h * 128:(h + 1) * 128]

    BT64 = (B_mat.T * SC).astype(np.float32)
    A1x64 = (pw[1] * SC).astype(np.float32)
    H64m = (H.T * SC).astype(np.float32)
    w1 = np.empty((128, 8 * 128), np.float32)
    for k in range(2):
        for h in range(2):
            w1[:, (k * 2 + h) * 128:(k * 2 + h + 1) * 128] = blk(BT64, k, h)
            t = 4 + k * 2 + h
            w1[:, t * 128:(t + 1) * 128] = blk(A1x64, k, h)
    w1 = w1.astype(NPBF)
    # fp8 DoubleRow stationaries: [p, tile, r, m]
    w8 = np.empty((128, 4, 2, 128), np.float32)
    for h in range(2):
        for r in range(2):
            w8[:, 0 + h, r, :] = SC * sqW.T[r * 128:(r + 1) * 128, h * 128:(h + 1) * 128]
            w8[:, 2 + h, r, :] = SC * sqV.T[r * 128:(r + 1) * 128, h * 128:(h + 1) * 128]
    w8 = w8.reshape(128, 1024).astype(NPF8)
    w2 = np.empty((128, 76 * 128), np.float32)
    for n in range(1, 17):
        for k in range(2):
            for h in range(2):
                t = (n - 1) * 4 + k * 2 + h
                w2[:, t * 128:(t + 1) * 128] = blk(pw[n], k, h)
    for base, mat in ((64, AC), (68, AC2), (72, H64m)):
        for k in range(2):
            for h in range(2):
                t = base + k * 2 + h
                w2[:, t * 128:(t + 1) * 128] = blk(mat, k, h)
    w2 = w2.astype(NPBF)

    in_maps = []
    for core in range(NCORE):
        sl = slice(core * B_loc, (core + 1) * B_loc)
        # [b, nch, j, ii, k, p] -> [j, p, ii, k, nch, b]
        xb = np.ascontiguousarray(
            inputs["inputs"][sl].reshape(B_loc, NCH, NJ, 2, 2, 128)
            .transpose(2, 5, 3, 4, 1, 0).reshape(NJ * 128, 1024)).astype(NPBF)
        xw = np.ascontiguousarray(
            inputs["W_noise"][sl].reshape(B_loc, NCH, NJ, 2, 2, 128)
            .transpose(2, 5, 3, 4, 1, 0).reshape(NJ * 128, 1024)).astype(NPF8)
        # [b, nch, j, ii, r, p] -> [j, p, r, ii, nch, b]
        vnb = np.ascontiguousarray(
            inputs["V_noise"][sl].reshape(B_loc, NCH, NJ, 2, 2, 128)
            .transpose(2, 5, 4, 3, 1, 0).reshape(NJ * 128, 1024)).astype(NPF8)
        s0T = np.ascontiguousarray(inputs["state0"][sl].T).astype(NPBF)
        in_maps.append(dict(xb=xb, xw=xw, vnb=vnb, s0t=s0T, w1=w1, w8=w8, w2=w2))

    if not _nc_cache:
        _nc_cache.append(_build())
    res = run_bass_kernel_spmd(_nc_cache[0], in_maps, core_ids=list(range(NCORE)),
                               trace=TRACE)
    last_result.clear()
    last_result.append(res)

    states = np.empty((B, T, S), np.float32)
    obs = np.empty((B, T, S), np.float32)
    for core, r in enumerate(res.results):
        sl = slice(core * B_loc, (core + 1) * B_loc)
        # [j2, h, p, q, nch, b] -> [b, nch, j2, q, h, p]
        states[sl] = (r["stb"].astype(np.float32).reshape(NJ // 2, 2, 128, 4, NCH, B_loc)
                      .transpose(5, 4, 0, 3, 1, 2).reshape(B_loc, T, S))
        # [j2, h, p, blk, ii, nch, b] -> [b, nch, j2, blk, ii, h, p]
        obs[sl] = (r["obb"].astype(np.float32).reshape(NJ // 2, 2, 128, 2, 2, NCH, B_loc)
                   .transpose(6, 5, 0, 3, 4, 1, 2).reshape(B_loc, T, S))
    return states, obs
